# revision 9
# baseline (speedup 1.0000x reference)
"""Trainium2 Bass kernel v7 — v6 plus PE offload of the double-cumsum for
tiles {0,4,8,12}. DVE: membrane scans (all tiles) + fp16 cumsums (12 tiles)
+ tiny stq copies. PE: per offloaded tile, 8 transposes + 8 S/A/B matmuls +
2 z-local matmuls + 7 exact-fp16 carry matmuls. Act: Sign+Relu spike compare,
membrane fp16 downcast, spkT/z PSUM downcasts. out=(z==1) on host.
"""

import os
import numpy as np
import ml_dtypes

B_FULL, C, T = 16, 1024, 1024
N_CORES = 8
B_SHARD = B_FULL // N_CORES
P = 128
NG = C // P
NCH = T // P
NITER = B_SHARD * NG  # 16
NBUF = 3
NPS = 2
PE_TILES = (1, 4, 7, 10, 13)
NPE = len(PE_TILES)

_PROGRAM_CACHE = {}
LAST_RESULTS = None


def _pe_idx(t):
    return PE_TILES.index(t) if t in PE_TILES else None


def _make_consts():
    s = np.arange(P)
    wz = (np.maximum(0, s[None, :] - s[:, None] + 1)
          * (s[:, None] <= s[None, :])).astype(np.float16)
    wst = np.zeros((P, NCH, 24), np.float16)
    for j in range(NCH):
        wst[:, j, 3 * j] = 1.0
        wst[:, j, 3 * j + 1] = s // 16
        wst[:, j, 3 * j + 2] = s % 16
    wc = np.zeros((24, NCH, P), np.float16)
    for j in range(1, NCH):
        for i in range(j):
            wc[3 * i, j, :] = (s + 1) + P * (j - i)
            wc[3 * i + 1, j, :] = -16.0
            wc[3 * i + 2, j, :] = -1.0
    ident = np.eye(P, dtype=np.float16)
    return ident, wz, wst, wc


def _build_program():
    import concourse.bass as bass
    from concourse import mybir
    from contextlib import ExitStack

    f32 = mybir.dt.float32
    bf16 = mybir.dt.bfloat16
    fp16 = mybir.dt.float16
    op = mybir.AluOpType
    fn = mybir.ActivationFunctionType

    nc = bass.Bass()

    cur_d = nc.declare_dram_parameter("current", [B_SHARD, C, T], f32, isOutput=False)
    beta_d = nc.declare_dram_parameter("beta_r", [P, NG], f32, isOutput=False)
    vinit_d = nc.declare_dram_parameter("vinit_r", [P, B_SHARD, NG], f32, isOutput=False)
    vth_d = nc.declare_dram_parameter("vth_r", [P, B_SHARD, NG], f32, isOutput=False)
    ident_d = nc.declare_dram_parameter("identc", [P, P], fp16, isOutput=False)
    wz_d = nc.declare_dram_parameter("wzc", [P, P], fp16, isOutput=False)
    wst_d = nc.declare_dram_parameter("wstc", [P, NCH, 24], fp16, isOutput=False)
    wc_d = nc.declare_dram_parameter("wcc", [24, NCH, P], fp16, isOutput=False)
    m_d = nc.declare_dram_parameter("m_fp16", [B_SHARD, C, T], fp16, isOutput=True)
    z_d = nc.declare_dram_parameter("z_bf16", [B_SHARD, C, T], bf16, isOutput=True)
    ztr_d = nc.declare_dram_parameter("z_tr", [NPE, P, NCH, P], bf16, isOutput=True)

    def iter_slices(i):
        b, g = divmod(i, NG)
        return b, g, g * P, (g + 1) * P

    # python-side store counters for slot-WAR waits
    zst_count = [0] * NBUF          # z stores per DVE-z slot
    dve_rank = {}
    r = 0
    for t in range(NITER):
        if t not in PE_TILES:
            dve_rank[t] = r
            r += 1

    with ExitStack() as st:
        block = st.enter_context(nc.Block())
        s_ld = st.enter_context(nc.semaphore("s_ld"))
        s_ldv = st.enter_context(nc.semaphore("s_ldv"))
        s_set = st.enter_context(nc.semaphore("s_set"))
        s_init = st.enter_context(nc.semaphore("s_init"))
        s_cur = [st.enter_context(nc.semaphore(f"s_cur{j}")) for j in range(NBUF)]
        s_mem = st.enter_context(nc.semaphore("s_mem"))
        s_sg = st.enter_context(nc.semaphore("s_sg"))
        s_rel = st.enter_context(nc.semaphore("s_rel"))
        s_mdc = st.enter_context(nc.semaphore("s_mdc"))
        s_c1 = st.enter_context(nc.semaphore("s_c1"))
        s_z = st.enter_context(nc.semaphore("s_z"))
        s_tr = st.enter_context(nc.semaphore("s_tr"))
        s_sdc = st.enter_context(nc.semaphore("s_sdc"))
        s_stm = st.enter_context(nc.semaphore("s_stm"))
        s_zlm = st.enter_context(nc.semaphore("s_zlm"))
        s_stq = st.enter_context(nc.semaphore("s_stq"))
        s_car = st.enter_context(nc.semaphore("s_car"))
        s_zdc = st.enter_context(nc.semaphore("s_zdc"))
        s_mst = [st.enter_context(nc.semaphore(f"s_mst{j}")) for j in range(NBUF)]
        s_zst = [st.enter_context(nc.semaphore(f"s_zst{j}")) for j in range(NBUF)]
        s_zpest = [st.enter_context(nc.semaphore(f"s_zpest{j}")) for j in range(NPS)]

        cur_sb = st.enter_context(nc.sbuf_tensor("cur_sb", [P, NBUF, T], f32))
        spk_sb = st.enter_context(nc.sbuf_tensor("spk_sb", [P, NBUF, T], fp16))
        t1_sb = st.enter_context(nc.sbuf_tensor("t1_sb", [P, 2, T], bf16))
        y_sb = st.enter_context(nc.sbuf_tensor("y_sb", [P, 2, T], fp16))
        z_sb = st.enter_context(nc.sbuf_tensor("z_sb", [P, NBUF, T], bf16))
        zpe_sb = st.enter_context(nc.sbuf_tensor("zpe_sb", [P, NPS, T], bf16))
        m_sb = st.enter_context(nc.sbuf_tensor("m_sb", [P, NBUF, T], fp16))
        spkT_sb = st.enter_context(nc.sbuf_tensor("spkT_sb", [P, NPS, NCH, P], fp16))
        stq_sb = st.enter_context(nc.sbuf_tensor("stq_sb", [24, NPS, P], fp16))
        beta_sb = st.enter_context(nc.sbuf_tensor("beta_sb", [P, NG], f32))
        vinit_sb = st.enter_context(nc.sbuf_tensor("vinit_sb", [P, B_SHARD, NG], f32))
        vth_sb = st.enter_context(nc.sbuf_tensor("vth_sb", [P, B_SHARD, NG], f32))
        nvth_sb = st.enter_context(nc.sbuf_tensor("nvth_sb", [P, B_SHARD, NG], f32))
        ones_sb = st.enter_context(nc.sbuf_tensor("ones_sb", [P, T], fp16))
        ident_sb = st.enter_context(nc.sbuf_tensor("ident_sb", [P, P], fp16))
        wz_sb = st.enter_context(nc.sbuf_tensor("wz_sb", [P, P], fp16))
        wst_sb = st.enter_context(nc.sbuf_tensor("wst_sb", [P, NCH, 24], fp16))
        wc_sb = st.enter_context(nc.sbuf_tensor("wc_sb", [24, NCH, P], fp16))

        ps_spkT = [nc.alloc_psum_tensor(f"ps_spkT{k}", [P, NCH, P], fp16) for k in range(NPS)]
        ps_z = [nc.alloc_psum_tensor(f"ps_z{k}", [P, NCH, P], f32) for k in range(NPS)]
        ps_st = [nc.alloc_psum_tensor(f"ps_st{k}", [24, P], f32) for k in range(NPS)]

        @block.sync
        def _(sp):
            sp.dma_start(out=cur_sb[:, 0, :], in_=cur_d[0, 0:P, :]).then_inc(s_cur[0], 16)
            sp.dma_start(out=beta_sb[:], in_=beta_d[:]).then_inc(s_ldv, 16)
            sp.dma_start(out=vinit_sb[:], in_=vinit_d[:]).then_inc(s_ldv, 16)
            sp.dma_start(out=vth_sb[:], in_=vth_d[:]).then_inc(s_ldv, 16)
            sp.dma_start(out=ident_sb[:], in_=ident_d[:]).then_inc(s_ld, 16)
            sp.dma_start(out=wz_sb[:], in_=wz_d[:]).then_inc(s_ld, 16)
            sp.dma_start(out=wst_sb[:], in_=wst_d[:]).then_inc(s_ld, 16)
            sp.dma_start(out=wc_sb[:], in_=wc_d[:]).then_inc(s_ld, 16)

            for i in range(1, NITER):
                b, g, c0, c1 = iter_slices(i)
                sl = i % NBUF
                if i >= NBUF:
                    sp.wait_ge(s_mdc, i - NBUF + 1)  # m-dc is the last cur reader
                sp.dma_start(out=cur_sb[:, sl, :], in_=cur_d[b, c0:c1, :]).then_inc(s_cur[sl], 16)

        @block.vector
        def _(vec):
            vec.memset(ones_sb[:], 1.0).then_inc(s_set, 1)
            vec.wait_ge(s_set, 1)
            vec.wait_ge(s_ldv, 3 * 16)
            vec.tensor_scalar(nvth_sb[:], vth_sb[:], -1.0, None, op.mult).then_inc(s_init, 1)
            vec.wait_ge(s_init, 1)

            zslot_prev_rank = {}  # slot -> rank of last DVE tile that used it
            yslot_prev_rank = {}

            def cschain(t):
                tsl = t % NBUF
                rk = dve_rank[t]
                ys = rk % 2
                vec.wait_ge(s_rel, t + 1)
                if ys in yslot_prev_rank:
                    vec.wait_ge(s_z, yslot_prev_rank[ys] + 1)  # y slot free
                yslot_prev_rank[ys] = rk
                vec.tensor_tensor_scan(
                    out=y_sb[:, ys, :], data0=ones_sb[:], data1=spk_sb[:, tsl, :],
                    initial=0.0, op0=op.mult, op1=op.add,
                ).then_inc(s_c1, 1)
                vec.wait_ge(s_c1, rk + 1)
                if tsl in zslot_prev_rank:
                    vec.wait_ge(s_zst[tsl], 16 * zslot_prev_rank[tsl])  # z slot free
                zslot_prev_rank[tsl] = zst_count_snapshot(tsl)
                vec.tensor_tensor_scan(
                    out=z_sb[:, tsl, :], data0=ones_sb[:], data1=y_sb[:, ys, :],
                    initial=0.0, op0=op.mult, op1=op.add,
                ).then_inc(s_z, 1)

            # snapshot helper: number of stores that will have been counted on
            # slot before this tile reuses it (python-side bookkeeping)
            zst_sched = [0] * NBUF
            store_order = []
            for i2 in range(NITER):
                if i2 >= 2 and (i2 - 2) not in PE_TILES:
                    store_order.append(i2 - 2)
            store_order += [NITER - 2, NITER - 1]
            seen = set()
            store_seq = []
            for t2 in store_order:
                if t2 not in seen and t2 not in PE_TILES:
                    seen.add(t2)
                    store_seq.append(t2)
            stores_before = {}
            cnt = [0] * NBUF
            for t2 in store_seq:
                stores_before[t2] = cnt[t2 % NBUF]
                cnt[t2 % NBUF] += 1

            def zst_count_snapshot(tsl):
                return 0  # replaced below

            # rebind cschain with correct counting
            def cschain(t):  # noqa: F811
                tsl = t % NBUF
                rk = dve_rank[t]
                ys = rk % 2
                vec.wait_ge(s_rel, t + 1)
                if rk >= 2:
                    # y slot used by DVE tile with rank rk-2
                    vec.wait_ge(s_z, rk - 1)
                vec.tensor_tensor_scan(
                    out=y_sb[:, ys, :], data0=ones_sb[:], data1=spk_sb[:, tsl, :],
                    initial=0.0, op0=op.mult, op1=op.add,
                ).then_inc(s_c1, 1)
                vec.wait_ge(s_c1, rk + 1)
                if stores_before[t] > 0:
                    vec.wait_ge(s_zst[tsl], 16 * stores_before[t])
                vec.tensor_tensor_scan(
                    out=z_sb[:, tsl, :], data0=ones_sb[:], data1=y_sb[:, ys, :],
                    initial=0.0, op0=op.mult, op1=op.add,
                ).then_inc(s_z, 1)

            for i in range(NITER):
                b, g, c0, c1 = iter_slices(i)
                sl = i % NBUF
                k = i // NBUF
                cur_t = cur_sb[:, sl, :]
                vec.wait_ge(s_cur[sl], 16 * (k + 1))
                vec.tensor_tensor_scan(
                    out=cur_t,
                    data0=beta_sb[:, g:g + 1].broadcast_to([P, T]),
                    data1=cur_t,
                    initial=vinit_sb[:, b, g:g + 1],
                    op0=op.mult, op1=op.add,
                ).then_inc(s_mem, 1)
                # stq copy for PE tile finished by PE st-matmuls
                stq_t = i - 3 if i == PE_TILES[0] + 3 else (i - 2 if i >= 2 and (i - 2) in PE_TILES and i - 2 != PE_TILES[0] else None)
                if stq_t is not None and stq_t in PE_TILES:
                    pk = _pe_idx(stq_t)
                    vec.wait_ge(s_stm, pk + 1)
                    vec.tensor_scalar(
                        stq_sb[:, pk % NPS, :], ps_st[pk % NPS][:], 1.0, None, op.mult
                    ).then_inc(s_stq, 1)
                if i >= 1 and (i - 1) not in PE_TILES:
                    cschain(i - 1)
            cschain(NITER - 1)

        @block.tensor
        def _(pe):
            pe.wait_ge(s_ld, 4 * 16)

            def matmuls_for(k):
                pps = k % NPS
                pe.wait_ge(s_sdc, k + 1)
                if k >= NPS:
                    pe.wait_ge(s_stq, k - NPS + 1)
                    pe.wait_ge(s_zdc, k - NPS + 1)
                for j in range(NCH):
                    mi = pe.matmul(
                        out=ps_st[pps][:], lhsT=wst_sb[:, j, :],
                        rhs=spkT_sb[:, pps, j, :],
                        start=(j == 0), stop=(j == NCH - 1), skip_group_check=True,
                    )
                mi.then_inc(s_stm, 1)
                pe.matmul(
                    out=ps_z[pps][:, 0:4, :].rearrange("p a b -> p (a b)"),
                    lhsT=wz_sb[:],
                    rhs=spkT_sb[:, pps, 0:4, :].rearrange("p a b -> p (a b)"),
                    start=True, stop=False, skip_group_check=True,
                )
                pe.matmul(
                    out=ps_z[pps][:, 4:8, :].rearrange("p a b -> p (a b)"),
                    lhsT=wz_sb[:],
                    rhs=spkT_sb[:, pps, 4:8, :].rearrange("p a b -> p (a b)"),
                    start=True, stop=False, skip_group_check=True,
                ).then_inc(s_zlm, 1)
                pe.wait_ge(s_stq, k + 1)
                for j in range(1, NCH):
                    mi = pe.matmul(
                        out=ps_z[pps][:, j, :],
                        lhsT=wc_sb[0:3 * j, j, :],
                        rhs=stq_sb[0:3 * j, pps, :],
                        start=False, stop=(j == NCH - 1), skip_group_check=True,
                    )
                mi.then_inc(s_car, 1)

            for k, t in enumerate(PE_TILES):
                ps = k % NPS
                sl = t % NBUF
                if k >= 1:
                    matmuls_for(k - 1)
                pe.wait_ge(s_rel, t + 1)
                if k >= NPS:
                    pe.wait_ge(s_sdc, k - NPS + 1)
                for j in range(NCH):
                    mi = pe.transpose(
                        out=ps_spkT[ps][:, j, :],
                        in_=spk_sb[:, sl, j * P:(j + 1) * P],
                        identity=ident_sb[:],
                    )
                mi.then_inc(s_tr, 1)
            matmuls_for(NPE - 1)

        @block.scalar
        def _(act):
            def store_m(t):
                tb, tg, tc0, tc1 = iter_slices(t)
                tsl = t % NBUF
                act.wait_ge(s_mdc, t + 1)
                act.dma_start(out=m_d[tb, tc0:tc1, :], in_=m_sb[:, tsl, :]).then_inc(s_mst[tsl], 16)

            def store_z_dve(t):
                tb, tg, tc0, tc1 = iter_slices(t)
                tsl = t % NBUF
                act.wait_ge(s_z, dve_rank[t] + 1)
                act.dma_start(out=z_d[tb, tc0:tc1, :], in_=z_sb[:, tsl, :]).then_inc(s_zst[tsl], 16)

            def store_z_pe(t):
                k = _pe_idx(t)
                act.wait_ge(s_zdc, k + 1)
                act.dma_start(
                    out=ztr_d[k].rearrange("t j c -> t (j c)"),
                    in_=zpe_sb[:, k % NPS, :],
                ).then_inc(s_zpest[k % NPS], 16)

            act.wait_ge(s_init, 1)
            for i in range(NITER):
                b, g, c0, c1 = iter_slices(i)
                sl = i % NBUF
                ts = i % 2
                k3 = i // NBUF
                act.wait_ge(s_mem, i + 1)
                if i >= 2:
                    act.wait_ge(s_rel, i - 1)
                act.activation(
                    out=t1_sb[:, ts, :], in_=cur_sb[:, sl, :],
                    func=fn.Sign, bias=nvth_sb[:, b, g:g + 1],
                ).then_inc(s_sg, 1)
                act.wait_ge(s_sg, i + 1)
                if i >= NBUF:
                    pv = i - NBUF
                    if pv in PE_TILES:
                        act.wait_ge(s_tr, _pe_idx(pv) + 1)
                    else:
                        act.wait_ge(s_c1, dve_rank[pv] + 1)
                act.activation(
                    out=spk_sb[:, sl, :], in_=t1_sb[:, ts, :], func=fn.Relu,
                ).then_inc(s_rel, 1)
                if i >= NBUF:
                    act.wait_ge(s_mst[sl], 16 * k3)
                act.activation(
                    out=m_sb[:, sl, :], in_=cur_sb[:, sl, :], func=fn.Copy,
                ).then_inc(s_mdc, 1)
                if i >= 2:
                    store_m(i - 2)
                    if (i - 2) not in PE_TILES:
                        store_z_dve(i - 2)
                zs_t = i - 4 if i == PE_TILES[0] + 4 else (i - 3 if i >= 3 and (i - 3) in PE_TILES and i - 3 != PE_TILES[0] else None)
                if zs_t is not None and zs_t in PE_TILES:
                    store_z_pe(zs_t)
                if i in PE_TILES:
                    k = _pe_idx(i)
                    ps = k % NPS
                    act.wait_ge(s_tr, k + 1)
                    if k >= NPS:
                        act.wait_ge(s_zlm, k - NPS + 1)
                    act.activation(
                        out=spkT_sb[:, ps, :, :].rearrange("p a b -> p (a b)"),
                        in_=ps_spkT[ps][:].rearrange("p a b -> p (a b)"),
                        func=fn.Copy,
                    ).then_inc(s_sdc, 1)
                zdc_t = i - 3 if i == PE_TILES[0] + 3 else (i - 2 if i >= 2 and (i - 2) in PE_TILES and i - 2 != PE_TILES[0] else None)
                if zdc_t is not None and zdc_t in PE_TILES:
                    k = _pe_idx(zdc_t)
                    pps = k % NPS
                    act.wait_ge(s_car, k + 1)
                    if k >= NPS:
                        act.wait_ge(s_zpest[pps], 16 * (k // NPS))
                    act.activation(
                        out=zpe_sb[:, pps, :],
                        in_=ps_z[pps][:].rearrange("p a b -> p (a b)"),
                        func=fn.Copy,
                    ).then_inc(s_zdc, 1)
            store_m(NITER - 2)
            store_z_dve(NITER - 2)
            store_m(NITER - 1)
            store_z_dve(NITER - 1)
            store_z_pe(PE_TILES[-1])

    return nc


def get_program():
    if "nc" not in _PROGRAM_CACHE:
        _PROGRAM_CACHE["nc"] = _build_program()
    return _PROGRAM_CACHE["nc"]


def _kernel_numpy(current, beta, v_init, v_th):
    cur = current.astype(np.float64).copy()
    cur[:, :, 0] += (beta[None, :] * v_init).astype(np.float32)
    m = np.empty_like(cur)
    state = np.zeros(cur.shape[:2])
    for t in range(cur.shape[2]):
        state = (beta[None, :] * state).astype(np.float32).astype(np.float64) + cur[:, :, t]
        state = state.astype(np.float32).astype(np.float64)
        m[:, :, t] = state
    spk = (m > v_th).astype(np.float64)
    z = np.cumsum(np.cumsum(spk, axis=-1), axis=-1)
    out = np.where(z == 1.0, 1.0, 0.0)
    return out.astype(np.float32), z.astype(np.float32), m.astype(np.float32)


def kernel(current, beta, v_init, v_th):
    global LAST_RESULTS
    from concourse.bass_utils import run_bass_kernel_spmd

    current = np.ascontiguousarray(current, dtype=np.float32)
    beta = np.ascontiguousarray(beta, dtype=np.float32)
    v_init = np.ascontiguousarray(v_init, dtype=np.float32)
    v_th = np.ascontiguousarray(v_th, dtype=np.float32)

    if not np.all(v_th == v_th[:, :, :1]):
        return _kernel_numpy(current, beta, v_init, v_th)

    nc = get_program()
    ident, wz, wst, wc = _make_consts()

    in_maps = []
    for k in range(N_CORES):
        lo, hi = k * B_SHARD, (k + 1) * B_SHARD
        in_maps.append({
            "current": np.ascontiguousarray(current[lo:hi]),
            "beta_r": np.ascontiguousarray(beta.reshape(NG, P).T),
            "vinit_r": np.ascontiguousarray(
                v_init[lo:hi].reshape(B_SHARD, NG, P).transpose(2, 0, 1)),
            "vth_r": np.ascontiguousarray(
                v_th[lo:hi, :, 0].reshape(B_SHARD, NG, P).transpose(2, 0, 1)),
            "identc": ident, "wzc": wz, "wstc": wst, "wcc": wc,
        })

    trace = bool(int(os.environ.get("KERNEL_TRACE", "0")))
    res = run_bass_kernel_spmd(nc, in_maps, list(range(N_CORES)), trace=trace)
    LAST_RESULTS = res

    outs, zs, ms = [], [], []
    for r in res.results:
        z = np.asarray(r["z_bf16"]).astype(np.float32)
        z_tr = np.asarray(r["z_tr"]).astype(np.float32)  # [NPE, t, j, c]
        for k, t in enumerate(PE_TILES):
            b, g = divmod(t, NG)
            z[b, g * P:(g + 1) * P, :] = (
                z_tr[k].transpose(2, 1, 0).reshape(P, T)
            )
        m = np.asarray(r["m_fp16"]).astype(np.float32)
        outs.append((z == 1.0).astype(np.float32))
        zs.append(z)
        ms.append(m)
    return (
        np.concatenate(outs, axis=0),
        np.concatenate(zs, axis=0),
        np.concatenate(ms, axis=0),
    )


# revision 10
# speedup vs baseline: 1.0128x; 1.0128x over previous
"""Trainium2 Bass kernel v7 — v6 plus PE offload of the double-cumsum for
tiles {0,4,8,12}. DVE: membrane scans (all tiles) + fp16 cumsums (12 tiles)
+ tiny stq copies. PE: per offloaded tile, 8 transposes + 8 S/A/B matmuls +
2 z-local matmuls + 7 exact-fp16 carry matmuls. Act: Sign+Relu spike compare,
membrane fp16 downcast, spkT/z PSUM downcasts. out=(z==1) on host.
"""

import os
import numpy as np
import ml_dtypes

B_FULL, C, T = 16, 1024, 1024
N_CORES = 8
B_SHARD = B_FULL // N_CORES
P = 128
NG = C // P
NCH = T // P
NITER = B_SHARD * NG  # 16
NBUF = 3
NPS = 2
PE_TILES = (0, 3, 6, 9, 12)
NPE = len(PE_TILES)

_PROGRAM_CACHE = {}
LAST_RESULTS = None


def _pe_idx(t):
    return PE_TILES.index(t) if t in PE_TILES else None


def _make_consts():
    s = np.arange(P)
    wz = (np.maximum(0, s[None, :] - s[:, None] + 1)
          * (s[:, None] <= s[None, :])).astype(np.float16)
    wst = np.zeros((P, NCH, 24), np.float16)
    for j in range(NCH):
        wst[:, j, 3 * j] = 1.0
        wst[:, j, 3 * j + 1] = s // 16
        wst[:, j, 3 * j + 2] = s % 16
    wc = np.zeros((24, NCH, P), np.float16)
    for j in range(1, NCH):
        for i in range(j):
            wc[3 * i, j, :] = (s + 1) + P * (j - i)
            wc[3 * i + 1, j, :] = -16.0
            wc[3 * i + 2, j, :] = -1.0
    ident = np.eye(P, dtype=np.float16)
    return ident, wz, wst, wc


def _build_program():
    import concourse.bass as bass
    from concourse import mybir
    from contextlib import ExitStack

    f32 = mybir.dt.float32
    bf16 = mybir.dt.bfloat16
    fp16 = mybir.dt.float16
    op = mybir.AluOpType
    fn = mybir.ActivationFunctionType

    nc = bass.Bass()

    cur_d = nc.declare_dram_parameter("current", [B_SHARD, C, T], f32, isOutput=False)
    beta_d = nc.declare_dram_parameter("beta_r", [P, NG], f32, isOutput=False)
    vinit_d = nc.declare_dram_parameter("vinit_r", [P, B_SHARD, NG], f32, isOutput=False)
    vth_d = nc.declare_dram_parameter("vth_r", [P, B_SHARD, NG], f32, isOutput=False)
    ident_d = nc.declare_dram_parameter("identc", [P, P], fp16, isOutput=False)
    wz_d = nc.declare_dram_parameter("wzc", [P, P], fp16, isOutput=False)
    wst_d = nc.declare_dram_parameter("wstc", [P, NCH, 24], fp16, isOutput=False)
    wc_d = nc.declare_dram_parameter("wcc", [24, NCH, P], fp16, isOutput=False)
    m_d = nc.declare_dram_parameter("m_fp16", [B_SHARD, C, T], fp16, isOutput=True)
    z_d = nc.declare_dram_parameter("z_bf16", [B_SHARD, C, T], bf16, isOutput=True)
    ztr_d = nc.declare_dram_parameter("z_tr", [NPE, P, NCH, P], bf16, isOutput=True)

    def iter_slices(i):
        b, g = divmod(i, NG)
        return b, g, g * P, (g + 1) * P

    # python-side store counters for slot-WAR waits
    zst_count = [0] * NBUF          # z stores per DVE-z slot
    dve_rank = {}
    r = 0
    for t in range(NITER):
        if t not in PE_TILES:
            dve_rank[t] = r
            r += 1

    with ExitStack() as st:
        block = st.enter_context(nc.Block())
        s_ld = st.enter_context(nc.semaphore("s_ld"))
        s_ldv = st.enter_context(nc.semaphore("s_ldv"))
        s_set = st.enter_context(nc.semaphore("s_set"))
        s_init = st.enter_context(nc.semaphore("s_init"))
        s_cur = [st.enter_context(nc.semaphore(f"s_cur{j}")) for j in range(NBUF)]
        s_mem = st.enter_context(nc.semaphore("s_mem"))
        s_sg = st.enter_context(nc.semaphore("s_sg"))
        s_rel = st.enter_context(nc.semaphore("s_rel"))
        s_mdc = st.enter_context(nc.semaphore("s_mdc"))
        s_c1 = st.enter_context(nc.semaphore("s_c1"))
        s_z = st.enter_context(nc.semaphore("s_z"))
        s_tr = st.enter_context(nc.semaphore("s_tr"))
        s_sdc = st.enter_context(nc.semaphore("s_sdc"))
        s_stm = st.enter_context(nc.semaphore("s_stm"))
        s_zlm = st.enter_context(nc.semaphore("s_zlm"))
        s_stq = st.enter_context(nc.semaphore("s_stq"))
        s_car = st.enter_context(nc.semaphore("s_car"))
        s_zdc = st.enter_context(nc.semaphore("s_zdc"))
        s_mst = [st.enter_context(nc.semaphore(f"s_mst{j}")) for j in range(NBUF)]
        s_zst = [st.enter_context(nc.semaphore(f"s_zst{j}")) for j in range(NBUF)]
        s_zpest = [st.enter_context(nc.semaphore(f"s_zpest{j}")) for j in range(NPS)]

        cur_sb = st.enter_context(nc.sbuf_tensor("cur_sb", [P, NBUF, T], f32))
        spk_sb = st.enter_context(nc.sbuf_tensor("spk_sb", [P, NBUF, T], fp16))
        t1_sb = st.enter_context(nc.sbuf_tensor("t1_sb", [P, 2, T], bf16))
        y_sb = st.enter_context(nc.sbuf_tensor("y_sb", [P, 2, T], fp16))
        z_sb = st.enter_context(nc.sbuf_tensor("z_sb", [P, NBUF, T], bf16))
        zpe_sb = st.enter_context(nc.sbuf_tensor("zpe_sb", [P, NPS, T], bf16))
        m_sb = st.enter_context(nc.sbuf_tensor("m_sb", [P, NBUF, T], fp16))
        spkT_sb = st.enter_context(nc.sbuf_tensor("spkT_sb", [P, NPS, NCH, P], fp16))
        stq_sb = st.enter_context(nc.sbuf_tensor("stq_sb", [24, NPS, P], fp16))
        beta_sb = st.enter_context(nc.sbuf_tensor("beta_sb", [P, NG], f32))
        vinit_sb = st.enter_context(nc.sbuf_tensor("vinit_sb", [P, B_SHARD, NG], f32))
        vth_sb = st.enter_context(nc.sbuf_tensor("vth_sb", [P, B_SHARD, NG], f32))
        nvth_sb = st.enter_context(nc.sbuf_tensor("nvth_sb", [P, B_SHARD, NG], f32))
        ones_sb = st.enter_context(nc.sbuf_tensor("ones_sb", [P, T], fp16))
        ident_sb = st.enter_context(nc.sbuf_tensor("ident_sb", [P, P], fp16))
        wz_sb = st.enter_context(nc.sbuf_tensor("wz_sb", [P, P], fp16))
        wst_sb = st.enter_context(nc.sbuf_tensor("wst_sb", [P, NCH, 24], fp16))
        wc_sb = st.enter_context(nc.sbuf_tensor("wc_sb", [24, NCH, P], fp16))

        ps_spkT = [nc.alloc_psum_tensor(f"ps_spkT{k}", [P, NCH, P], fp16) for k in range(NPS)]
        ps_z = [nc.alloc_psum_tensor(f"ps_z{k}", [P, NCH, P], f32) for k in range(NPS)]
        ps_st = [nc.alloc_psum_tensor(f"ps_st{k}", [24, P], f32) for k in range(NPS)]

        @block.sync
        def _(sp):
            sp.dma_start(out=cur_sb[:, 0, :], in_=cur_d[0, 0:P, :]).then_inc(s_cur[0], 16)
            sp.dma_start(out=beta_sb[:], in_=beta_d[:]).then_inc(s_ldv, 16)
            sp.dma_start(out=vinit_sb[:], in_=vinit_d[:]).then_inc(s_ldv, 16)
            sp.dma_start(out=vth_sb[:], in_=vth_d[:]).then_inc(s_ldv, 16)
            sp.dma_start(out=ident_sb[:], in_=ident_d[:]).then_inc(s_ld, 16)
            sp.dma_start(out=wz_sb[:], in_=wz_d[:]).then_inc(s_ld, 16)
            sp.dma_start(out=wst_sb[:], in_=wst_d[:]).then_inc(s_ld, 16)
            sp.dma_start(out=wc_sb[:], in_=wc_d[:]).then_inc(s_ld, 16)

            for i in range(1, NITER):
                b, g, c0, c1 = iter_slices(i)
                sl = i % NBUF
                if i >= NBUF:
                    sp.wait_ge(s_mdc, i - NBUF + 1)  # m-dc is the last cur reader
                sp.dma_start(out=cur_sb[:, sl, :], in_=cur_d[b, c0:c1, :]).then_inc(s_cur[sl], 16)

        @block.vector
        def _(vec):
            vec.memset(ones_sb[:], 1.0).then_inc(s_set, 1)
            vec.wait_ge(s_set, 1)
            vec.wait_ge(s_ldv, 3 * 16)
            vec.tensor_scalar(nvth_sb[:], vth_sb[:], -1.0, None, op.mult).then_inc(s_init, 1)
            vec.wait_ge(s_init, 1)

            zslot_prev_rank = {}  # slot -> rank of last DVE tile that used it
            yslot_prev_rank = {}

            def cschain(t):
                tsl = t % NBUF
                rk = dve_rank[t]
                ys = rk % 2
                vec.wait_ge(s_rel, t + 1)
                if ys in yslot_prev_rank:
                    vec.wait_ge(s_z, yslot_prev_rank[ys] + 1)  # y slot free
                yslot_prev_rank[ys] = rk
                vec.tensor_tensor_scan(
                    out=y_sb[:, ys, :], data0=ones_sb[:], data1=spk_sb[:, tsl, :],
                    initial=0.0, op0=op.mult, op1=op.add,
                ).then_inc(s_c1, 1)
                vec.wait_ge(s_c1, rk + 1)
                if tsl in zslot_prev_rank:
                    vec.wait_ge(s_zst[tsl], 16 * zslot_prev_rank[tsl])  # z slot free
                zslot_prev_rank[tsl] = zst_count_snapshot(tsl)
                vec.tensor_tensor_scan(
                    out=z_sb[:, tsl, :], data0=ones_sb[:], data1=y_sb[:, ys, :],
                    initial=0.0, op0=op.mult, op1=op.add,
                ).then_inc(s_z, 1)

            # snapshot helper: number of stores that will have been counted on
            # slot before this tile reuses it (python-side bookkeeping)
            zst_sched = [0] * NBUF
            store_order = []
            for i2 in range(NITER):
                if i2 >= 2 and (i2 - 2) not in PE_TILES:
                    store_order.append(i2 - 2)
            store_order += [NITER - 2, NITER - 1]
            seen = set()
            store_seq = []
            for t2 in store_order:
                if t2 not in seen and t2 not in PE_TILES:
                    seen.add(t2)
                    store_seq.append(t2)
            stores_before = {}
            cnt = [0] * NBUF
            for t2 in store_seq:
                stores_before[t2] = cnt[t2 % NBUF]
                cnt[t2 % NBUF] += 1

            def zst_count_snapshot(tsl):
                return 0  # replaced below

            # rebind cschain with correct counting
            def cschain(t):  # noqa: F811
                tsl = t % NBUF
                rk = dve_rank[t]
                ys = rk % 2
                vec.wait_ge(s_rel, t + 1)
                if rk >= 2:
                    # y slot used by DVE tile with rank rk-2
                    vec.wait_ge(s_z, rk - 1)
                vec.tensor_tensor_scan(
                    out=y_sb[:, ys, :], data0=ones_sb[:], data1=spk_sb[:, tsl, :],
                    initial=0.0, op0=op.mult, op1=op.add,
                ).then_inc(s_c1, 1)
                vec.wait_ge(s_c1, rk + 1)
                if stores_before[t] > 0:
                    vec.wait_ge(s_zst[tsl], 16 * stores_before[t])
                vec.tensor_tensor_scan(
                    out=z_sb[:, tsl, :], data0=ones_sb[:], data1=y_sb[:, ys, :],
                    initial=0.0, op0=op.mult, op1=op.add,
                ).then_inc(s_z, 1)

            for i in range(NITER):
                b, g, c0, c1 = iter_slices(i)
                sl = i % NBUF
                k = i // NBUF
                cur_t = cur_sb[:, sl, :]
                vec.wait_ge(s_cur[sl], 16 * (k + 1))
                vec.tensor_tensor_scan(
                    out=cur_t,
                    data0=beta_sb[:, g:g + 1].broadcast_to([P, T]),
                    data1=cur_t,
                    initial=vinit_sb[:, b, g:g + 1],
                    op0=op.mult, op1=op.add,
                ).then_inc(s_mem, 1)
                # stq copy for PE tile finished by PE st-matmuls
                stq_t = i - 3 if i == 3 else (i - 2 if i >= 2 and (i - 2) in PE_TILES and i - 2 != 0 else None)
                if stq_t is not None and stq_t in PE_TILES:
                    pk = _pe_idx(stq_t)
                    vec.wait_ge(s_stm, pk + 1)
                    vec.tensor_scalar(
                        stq_sb[:, pk % NPS, :], ps_st[pk % NPS][:], 1.0, None, op.mult
                    ).then_inc(s_stq, 1)
                if i >= 1 and (i - 1) not in PE_TILES:
                    cschain(i - 1)
            cschain(NITER - 1)

        @block.tensor
        def _(pe):
            pe.wait_ge(s_ld, 4 * 16)

            def matmuls_for(k):
                pps = k % NPS
                pe.wait_ge(s_sdc, k + 1)
                if k >= NPS:
                    pe.wait_ge(s_stq, k - NPS + 1)
                    pe.wait_ge(s_zdc, k - NPS + 1)
                for j in range(NCH):
                    mi = pe.matmul(
                        out=ps_st[pps][:], lhsT=wst_sb[:, j, :],
                        rhs=spkT_sb[:, pps, j, :],
                        start=(j == 0), stop=(j == NCH - 1), skip_group_check=True,
                    )
                mi.then_inc(s_stm, 1)
                pe.matmul(
                    out=ps_z[pps][:, 0:4, :].rearrange("p a b -> p (a b)"),
                    lhsT=wz_sb[:],
                    rhs=spkT_sb[:, pps, 0:4, :].rearrange("p a b -> p (a b)"),
                    start=True, stop=False, skip_group_check=True,
                )
                pe.matmul(
                    out=ps_z[pps][:, 4:8, :].rearrange("p a b -> p (a b)"),
                    lhsT=wz_sb[:],
                    rhs=spkT_sb[:, pps, 4:8, :].rearrange("p a b -> p (a b)"),
                    start=True, stop=False, skip_group_check=True,
                ).then_inc(s_zlm, 1)
                pe.wait_ge(s_stq, k + 1)
                for j in range(1, NCH):
                    mi = pe.matmul(
                        out=ps_z[pps][:, j, :],
                        lhsT=wc_sb[0:3 * j, j, :],
                        rhs=stq_sb[0:3 * j, pps, :],
                        start=False, stop=(j == NCH - 1), skip_group_check=True,
                    )
                mi.then_inc(s_car, 1)

            for k, t in enumerate(PE_TILES):
                ps = k % NPS
                sl = t % NBUF
                if k >= 1:
                    matmuls_for(k - 1)
                pe.wait_ge(s_rel, t + 1)
                if k >= NPS:
                    pe.wait_ge(s_sdc, k - NPS + 1)
                for j in range(NCH):
                    mi = pe.transpose(
                        out=ps_spkT[ps][:, j, :],
                        in_=spk_sb[:, sl, j * P:(j + 1) * P],
                        identity=ident_sb[:],
                    )
                mi.then_inc(s_tr, 1)
            matmuls_for(NPE - 1)

        @block.scalar
        def _(act):
            def store_m(t):
                tb, tg, tc0, tc1 = iter_slices(t)
                tsl = t % NBUF
                act.wait_ge(s_mdc, t + 1)
                act.dma_start(out=m_d[tb, tc0:tc1, :], in_=m_sb[:, tsl, :]).then_inc(s_mst[tsl], 16)

            def store_z_dve(t):
                tb, tg, tc0, tc1 = iter_slices(t)
                tsl = t % NBUF
                act.wait_ge(s_z, dve_rank[t] + 1)
                act.dma_start(out=z_d[tb, tc0:tc1, :], in_=z_sb[:, tsl, :]).then_inc(s_zst[tsl], 16)

            def store_z_pe(t):
                k = _pe_idx(t)
                act.wait_ge(s_zdc, k + 1)
                act.dma_start(
                    out=ztr_d[k].rearrange("t j c -> t (j c)"),
                    in_=zpe_sb[:, k % NPS, :],
                ).then_inc(s_zpest[k % NPS], 16)

            act.wait_ge(s_init, 1)
            for i in range(NITER):
                b, g, c0, c1 = iter_slices(i)
                sl = i % NBUF
                ts = i % 2
                k3 = i // NBUF
                act.wait_ge(s_mem, i + 1)
                if i >= 2:
                    act.wait_ge(s_rel, i - 1)
                act.activation(
                    out=t1_sb[:, ts, :], in_=cur_sb[:, sl, :],
                    func=fn.Sign, bias=nvth_sb[:, b, g:g + 1],
                ).then_inc(s_sg, 1)
                act.wait_ge(s_sg, i + 1)
                if i >= NBUF:
                    pv = i - NBUF
                    if pv in PE_TILES:
                        act.wait_ge(s_tr, _pe_idx(pv) + 1)
                    else:
                        act.wait_ge(s_c1, dve_rank[pv] + 1)
                act.activation(
                    out=spk_sb[:, sl, :], in_=t1_sb[:, ts, :], func=fn.Relu,
                ).then_inc(s_rel, 1)
                if i >= NBUF:
                    act.wait_ge(s_mst[sl], 16 * k3)
                act.activation(
                    out=m_sb[:, sl, :], in_=cur_sb[:, sl, :], func=fn.Copy,
                ).then_inc(s_mdc, 1)
                if i >= 2:
                    store_m(i - 2)
                    if (i - 2) not in PE_TILES:
                        store_z_dve(i - 2)
                zs_t = i - 4 if i == 4 else (i - 3 if i >= 3 and (i - 3) in PE_TILES and i - 3 != 0 else None)
                if zs_t is not None and zs_t in PE_TILES:
                    store_z_pe(zs_t)
                if i in PE_TILES:
                    k = _pe_idx(i)
                    ps = k % NPS
                    act.wait_ge(s_tr, k + 1)
                    if k >= NPS:
                        act.wait_ge(s_zlm, k - NPS + 1)
                    act.activation(
                        out=spkT_sb[:, ps, :, :].rearrange("p a b -> p (a b)"),
                        in_=ps_spkT[ps][:].rearrange("p a b -> p (a b)"),
                        func=fn.Copy,
                    ).then_inc(s_sdc, 1)
                zdc_t = i - 3 if i == 3 else (i - 2 if i >= 2 and (i - 2) in PE_TILES and i - 2 != 0 else None)
                if zdc_t is not None and zdc_t in PE_TILES:
                    k = _pe_idx(zdc_t)
                    pps = k % NPS
                    act.wait_ge(s_car, k + 1)
                    if k >= NPS:
                        act.wait_ge(s_zpest[pps], 16 * (k // NPS))
                    act.activation(
                        out=zpe_sb[:, pps, :],
                        in_=ps_z[pps][:].rearrange("p a b -> p (a b)"),
                        func=fn.Copy,
                    ).then_inc(s_zdc, 1)
            store_m(NITER - 2)
            store_z_dve(NITER - 2)
            store_m(NITER - 1)
            store_z_dve(NITER - 1)

    return nc


def get_program():
    if "nc" not in _PROGRAM_CACHE:
        _PROGRAM_CACHE["nc"] = _build_program()
    return _PROGRAM_CACHE["nc"]


def _kernel_numpy(current, beta, v_init, v_th):
    cur = current.astype(np.float64).copy()
    cur[:, :, 0] += (beta[None, :] * v_init).astype(np.float32)
    m = np.empty_like(cur)
    state = np.zeros(cur.shape[:2])
    for t in range(cur.shape[2]):
        state = (beta[None, :] * state).astype(np.float32).astype(np.float64) + cur[:, :, t]
        state = state.astype(np.float32).astype(np.float64)
        m[:, :, t] = state
    spk = (m > v_th).astype(np.float64)
    z = np.cumsum(np.cumsum(spk, axis=-1), axis=-1)
    out = np.where(z == 1.0, 1.0, 0.0)
    return out.astype(np.float32), z.astype(np.float32), m.astype(np.float32)


def kernel(current, beta, v_init, v_th):
    global LAST_RESULTS
    from concourse.bass_utils import run_bass_kernel_spmd

    current = np.ascontiguousarray(current, dtype=np.float32)
    beta = np.ascontiguousarray(beta, dtype=np.float32)
    v_init = np.ascontiguousarray(v_init, dtype=np.float32)
    v_th = np.ascontiguousarray(v_th, dtype=np.float32)

    if not np.all(v_th == v_th[:, :, :1]):
        return _kernel_numpy(current, beta, v_init, v_th)

    nc = get_program()
    ident, wz, wst, wc = _make_consts()

    in_maps = []
    for k in range(N_CORES):
        lo, hi = k * B_SHARD, (k + 1) * B_SHARD
        in_maps.append({
            "current": np.ascontiguousarray(current[lo:hi]),
            "beta_r": np.ascontiguousarray(beta.reshape(NG, P).T),
            "vinit_r": np.ascontiguousarray(
                v_init[lo:hi].reshape(B_SHARD, NG, P).transpose(2, 0, 1)),
            "vth_r": np.ascontiguousarray(
                v_th[lo:hi, :, 0].reshape(B_SHARD, NG, P).transpose(2, 0, 1)),
            "identc": ident, "wzc": wz, "wstc": wst, "wcc": wc,
        })

    trace = bool(int(os.environ.get("KERNEL_TRACE", "0")))
    res = run_bass_kernel_spmd(nc, in_maps, list(range(N_CORES)), trace=trace)
    LAST_RESULTS = res

    outs, zs, ms = [], [], []
    for r in res.results:
        z = np.asarray(r["z_bf16"]).astype(np.float32)
        z_tr = np.asarray(r["z_tr"]).astype(np.float32)  # [NPE, t, j, c]
        for k, t in enumerate(PE_TILES):
            b, g = divmod(t, NG)
            z[b, g * P:(g + 1) * P, :] = (
                z_tr[k].transpose(2, 1, 0).reshape(P, T)
            )
        m = np.asarray(r["m_fp16"]).astype(np.float32)
        outs.append((z == 1.0).astype(np.float32))
        zs.append(z)
        ms.append(m)
    return (
        np.concatenate(outs, axis=0),
        np.concatenate(zs, axis=0),
        np.concatenate(ms, axis=0),
    )
